# revision 15
# baseline (speedup 1.0000x reference)
"""HVAE ELBO kernel for 8 Trainium2 NeuronCores.

Strategy: pure data-parallel over batch (2048 -> 8 x 256). All layers
(convs included) are applied as banded-dense matmuls on feature-major
activations [F, B] with B=256 as the moving free dim. Dense layer
matrices are built on the host by probing the jax conv ops with basis
vectors (exact), permuted to an (h, w, c) feature order, pruned to
nonzero 128x128 tiles, and shipped to SBUF as bf16. PSUM accumulates in
fp32. All transcendentals use only {Exp, Ln, Copy} so the ACT engine
loads a single table set. The leapfrog dU is evaluated K+1=6 times
(endpoint caching) instead of 2K=10. Backward weight matrices for the
two largest layers (D1T, D2T) are streamed from HBM per use; everything
else is SBUF-resident. Final ELBO reduction over features happens
on-chip via ones-vector matmuls; the batch mean and neg-KL assembly
happen on the host in float64.
"""

import sys

sys.path.insert(0, "/opt/trn_rl_repo")

import numpy as np
import ml_dtypes

Z_DIM = 20
K_LF = 5
MAX_LF = 0.5
B_FULL = 2048
N_CORES = 8
BC = B_FULL // N_CORES  # 256 per core

BF16 = ml_dtypes.bfloat16

_CACHE = {}


# ----------------------------------------------------------------------------
# host-side dense layer matrices (exact, via jax CPU probing)
# ----------------------------------------------------------------------------

def _hwc_perm(C, H, W):
    """perm[f_mine] = f_ref.  mine order (h,w,c); ref order (c,h,w)."""
    hh, ww, cc = np.meshgrid(
        np.arange(H), np.arange(W), np.arange(C), indexing="ij"
    )
    return ((cc * H + hh) * W + ww).reshape(-1)


def _build_mats(ins):
    import jax
    import jax.numpy as jnp
    from jax import lax

    cpu = jax.devices("cpu")[0]
    DN = ("NCHW", "OIHW", "NCHW")

    def conv(x, w, stride):
        return lax.conv_general_dilated(
            x, w, (stride, stride), [(2, 2), (2, 2)], dimension_numbers=DN
        )

    def conv_t(x, w, stride, out_pad):
        k = w.shape[2]
        lo, hi = k - 1 - 2, k - 1 - 2 + out_pad
        wt = jnp.flip(w, (2, 3)).transpose(1, 0, 2, 3)
        return lax.conv_general_dilated(
            x, wt, (1, 1), [(lo, hi), (lo, hi)],
            lhs_dilation=(stride, stride), dimension_numbers=DN,
        )

    def probe(fn, C, H, W):
        basis = np.eye(C * H * W, dtype=np.float32).reshape(-1, C, H, W)
        out = np.asarray(fn(jnp.asarray(basis)))
        return out.reshape(out.shape[0], -1)  # [Fin_ref, Fout_ref]

    mats = {}
    with jax.default_device(cpu):
        # encoder convs
        m = probe(lambda x: conv(x, ins["c1w"], 2), 1, 28, 28)
        mats["A1"] = m[:, _hwc_perm(16, 14, 14)].copy()  # in (h,w) natural
        m = probe(lambda x: conv(x, ins["c2w"], 2), 16, 14, 14)
        mats["A2"] = m[np.ix_(_hwc_perm(16, 14, 14), _hwc_perm(32, 7, 7))]
        m = probe(lambda x: conv(x, ins["c3w"], 1), 32, 7, 7)
        mats["A3"] = m[np.ix_(_hwc_perm(32, 7, 7), _hwc_perm(32, 7, 7))]
        # generator deconvs
        m = probe(lambda x: conv_t(x, ins["d1w"], 1, 0), 32, 7, 7)
        mats["D1"] = m[np.ix_(_hwc_perm(32, 7, 7), _hwc_perm(32, 7, 7))]
        m = probe(lambda x: conv_t(x, ins["d2w"], 2, 1), 32, 7, 7)
        mats["D2"] = m[np.ix_(_hwc_perm(32, 7, 7), _hwc_perm(16, 14, 14))]
        m = probe(lambda x: conv_t(x, ins["d3w"], 2, 1), 16, 14, 14)
        mats["D3"] = m[_hwc_perm(16, 14, 14), :].copy()  # out (h,w) natural

    p77 = _hwc_perm(32, 7, 7)
    mats["FC1"] = np.ascontiguousarray(ins["fc1_w"].T[p77, :])
    mats["FMU"] = np.ascontiguousarray(ins["fmu_w"].T)
    mats["FSIG"] = np.ascontiguousarray(ins["fsig_w"].T)
    mats["G1"] = np.ascontiguousarray(ins["g1w"].T)
    mats["G2"] = np.ascontiguousarray(ins["g2w"].T[:, p77])
    mats["D3T"] = np.ascontiguousarray(mats["D3"].T)
    mats["D2T"] = np.ascontiguousarray(mats["D2"].T)
    mats["D1T"] = np.ascontiguousarray(mats["D1"].T)
    mats["G2T"] = np.ascontiguousarray(mats["G2"].T)
    mats["G1T"] = np.ascontiguousarray(mats["G1"].T)

    def expand_bias(b, C, H, W):
        ref = np.repeat(np.asarray(b, np.float32), H * W)  # (c,h,w) order
        return ref[_hwc_perm(C, H, W)]

    biases = {
        "A1": expand_bias(ins["c1b"], 16, 14, 14),
        "A2": expand_bias(ins["c2b"], 32, 7, 7),
        "A3": expand_bias(ins["c3b"], 32, 7, 7),
        "FC1": np.asarray(ins["fc1_b"], np.float32),
        "FMU": np.asarray(ins["fmu_b"], np.float32),
        "FSIG": np.asarray(ins["fsig_b"], np.float32),
        "G1": np.asarray(ins["g1b"], np.float32),
        "G2": np.asarray(ins["g2b"], np.float32)[p77],
        "D1": expand_bias(ins["d1b"], 32, 7, 7),
        "D2": expand_bias(ins["d2b"], 16, 14, 14),
        "D3": np.repeat(np.asarray(ins["d3b"], np.float32), 784),
    }
    return mats, biases


def _tile_meta(mat):
    """-> (Kt, Mt, jobs) ; jobs[m] = (m_act, [kt,...]) over nonzero tiles."""
    Fin, Fout = mat.shape
    Kt, Mt = -(-Fin // 128), -(-Fout // 128)
    P = np.zeros((Kt * 128, Mt * 128), np.float32)
    P[:Fin, :Fout] = mat
    nz = np.abs(P).reshape(Kt, 128, Mt, 128).max(axis=(1, 3)) > 0
    jobs = []
    for m in range(Mt):
        m_act = min(128, Fout - m * 128)
        ks = [k for k in range(Kt) if nz[k, m]]
        jobs.append((m_act, ks))
    return Kt, Mt, jobs, P


class LayerPack:
    """Packs nonzero bf16 tiles column-major-by-m into one [128, n*128]."""

    def __init__(self, name, mat):
        self.name = name
        Kt, Mt, jobs, P = _tile_meta(mat)
        self.Kt, self.Mt = Kt, Mt
        cols = []
        self.jobs = []  # [(m_act, [(kt, col)...], m_col0, nk)]
        for m, (m_act, ks) in enumerate(jobs):
            c0 = len(cols)
            entry = []
            for kt in ks:
                entry.append((kt, len(cols)))
                cols.append(P[kt * 128:(kt + 1) * 128, m * 128:(m + 1) * 128])
            self.jobs.append((m_act, entry, c0, len(ks)))
        self.ncols = len(cols)
        self.max_nk = max(nk for _, _, _, nk in self.jobs)
        self.arena = np.concatenate(cols, axis=1).astype(BF16)


# ----------------------------------------------------------------------------
# bass kernel builder
# ----------------------------------------------------------------------------

def _build_kernel(packs, biases_vec, scal):
    import concourse.bass as bass
    import concourse.bacc as bacc
    import concourse.mybir as mybir
    import concourse.tile as tile
    from contextlib import ExitStack

    F32 = mybir.dt.float32
    BF = mybir.dt.bfloat16
    AF = mybir.ActivationFunctionType
    ALU = mybir.AluOpType

    nc = bacc.Bacc("TRN2", target_bir_lowering=False, debug=False)

    dram = {}
    for nm, pk in packs.items():
        dram[nm] = nc.dram_tensor(
            f"w_{nm}", [128, pk.ncols * 128], BF, kind="ExternalInput"
        ).ap()
    NBIAS = biases_vec.shape[1]
    d_bias = nc.dram_tensor("biases", [128, NBIAS], F32, kind="ExternalInput").ap()
    d_x = nc.dram_tensor("xin", [128, 7 * BC], BF, kind="ExternalInput").ap()
    d_epz = nc.dram_tensor("epz", [Z_DIM, BC], F32, kind="ExternalInput").ap()
    d_epp = nc.dram_tensor("epp", [Z_DIM, BC], F32, kind="ExternalInput").ap()
    d_lf = nc.dram_tensor("lfv", [Z_DIM, 2], F32, kind="ExternalInput").ap()
    d_out = nc.dram_tensor("out", [61, BC], F32, kind="ExternalOutput").ap()

    alphas = scal["alphas"]  # python floats, len K

    RES = ["G1", "G2", "D1", "D2", "D3", "D3T", "G2T", "G1T"]
    ENC = ["A1", "A2", "A3", "FC1", "FMU", "FSIG"]
    STR = ["D2T", "D1T"]
    bias_col = scal["bias_col"]  # name -> starting col in bias arena

    with tile.TileContext(nc) as tc, ExitStack() as ctx:
        psum = ctx.enter_context(tc.tile_pool(name="psum", bufs=6, space="PSUM"))
        pred = ctx.enter_context(tc.tile_pool(name="pred", bufs=2, space="PSUM"))
        pers = ctx.enter_context(tc.tile_pool(name="pers", bufs=1))
        tmp = ctx.enter_context(tc.tile_pool(name="tmp", bufs=4))
        zp = ctx.enter_context(tc.tile_pool(name="zp", bufs=3))

        # --- persistent SBUF tensors ---
        def pt(name, ntile, dt=BF):
            t = pers.tile([128, ntile * BC], dt, tag=name, name=name)
            nc.vector.memset(t[:], 0.0)
            return t

        a1, a2, a3, a4 = pt("a1", 4), pt("a2", 13), pt("a3", 13), pt("a4", 25)
        x_sb = pers.tile([128, 7 * BC], BF, tag="x")
        sd = pt("sd", 7)
        eL = pt("eL", 7)
        logit = pt("logit", 7)
        spl = pt("spl", 7)
        ell = pers.tile([128, BC], F32, tag="ell")
        bias_sb = pers.tile([128, NBIAS], F32, tag="bias")
        ones = pers.tile([128, 1], BF, tag="ones")
        nc.vector.memset(ones[:], 1.0)
        lf_sb = pers.tile([Z_DIM, 2], F32, tag="lf")  # col0 lf, col1 -lf/2
        mu = pers.tile([Z_DIM, BC], F32, tag="mu")
        sig = pers.tile([Z_DIM, BC], F32, tag="sig")
        lsig = pers.tile([Z_DIM, BC], F32, tag="lsig")

        nc.sync.dma_start(bias_sb[:], d_bias)
        nc.sync.dma_start(x_sb[:], d_x)
        nc.sync.dma_start(lf_sb[:], d_lf)

        w_sb = {}

        def apply_layer(nm, wt, src, epilogue, k_rows=128):
            pk = packs[nm]
            for m, (m_act, entry, _, _) in enumerate(pk.jobs):
                ps = psum.tile([128, BC], F32, tag="mm")
                for i, (kt, col) in enumerate(entry):
                    nc.tensor.matmul(
                        ps[0:m_act, :],
                        wt[0:k_rows, col * 128: col * 128 + m_act],
                        src[0:k_rows, kt * BC:(kt + 1) * BC],
                        start=(i == 0),
                        stop=(i == len(entry) - 1),
                    )
                epilogue(m, m_act, ps)

        def softplus_epi(nm, dst):
            c0 = bias_col[nm]

            def epi(m, m_act, ps):
                t = tmp.tile([128, BC], F32, tag="sp")
                nc.scalar.activation(
                    t[0:m_act, :], ps[0:m_act, :], AF.Exp,
                    bias=bias_sb[0:m_act, c0 + m: c0 + m + 1], scale=1.0,
                )
                nc.scalar.activation(
                    dst[0:m_act, m * BC: m * BC + BC], t[0:m_act, :],
                    AF.Ln, bias=1.0, scale=1.0,
                )
            return epi

        def stream_layer(nm, src, epilogue):
            pk = packs[nm]
            for m, (m_act, entry, c0, nk) in enumerate(pk.jobs):
                st = strm.tile([128, pk.max_nk * 128], BF, tag=f"st{nm}", name=f"st{nm}")
                nc.sync.dma_start(
                    st[:, 0: nk * 128], dram[nm][:, c0 * 128:(c0 + nk) * 128]
                )
                ps = psum.tile([128, BC], F32, tag="mm")
                for i, (kt, col) in enumerate(entry):
                    j = col - c0
                    nc.tensor.matmul(
                        ps[0:m_act, :],
                        st[:, j * 128: j * 128 + m_act],
                        src[:, kt * BC:(kt + 1) * BC],
                        start=(i == 0),
                        stop=(i == len(entry) - 1),
                    )
                epilogue(m, m_act, ps)

        # gp = (e - 1) * g  computed into dst (in-place over e is allowed)
        def grad_site_epi(e_src, dst):
            def epi(m, m_act, ps):
                nc.vector.scalar_tensor_tensor(
                    dst[0:m_act, m * BC: m * BC + BC],
                    e_src[0:m_act, m * BC: m * BC + BC],
                    1.0,
                    ps[0:m_act, :],
                    ALU.subtract,
                    ALU.mult,
                )
            return epi

        # ---------------- encoder ----------------
        with ExitStack() as enc_ctx:
            wenc = enc_ctx.enter_context(tc.tile_pool(name="wenc", bufs=1))
            eact = enc_ctx.enter_context(tc.tile_pool(name="eact", bufs=1))
            we = {}
            for nm in ENC:
                we[nm] = wenc.tile([128, packs[nm].ncols * 128], BF, tag=f"w{nm}", name=f"we{nm}")
                nc.sync.dma_start(we[nm][:], dram[nm])
            e1 = eact.tile([128, 25 * BC], BF, tag="e1")
            e2 = eact.tile([128, 13 * BC], BF, tag="e2")
            e3 = eact.tile([128, 13 * BC], BF, tag="e3")
            e4 = eact.tile([128, 4 * BC], BF, tag="e4")
            for t, n in ((e1, 25), (e2, 13), (e3, 13), (e4, 4)):
                nc.vector.memset(t[:], 0.0)

            apply_layer("A1", we["A1"], x_sb, softplus_epi("A1", e1))
            apply_layer("A2", we["A2"], e1, softplus_epi("A2", e2))
            apply_layer("A3", we["A3"], e2, softplus_epi("A3", e3))
            apply_layer("FC1", we["FC1"], e3, softplus_epi("FC1", e4))

            def mu_epi(m, m_act, ps):
                nc.scalar.activation(
                    mu[0:Z_DIM, :], ps[0:Z_DIM, :], AF.Identity,
                    bias=bias_sb[0:Z_DIM, bias_col["FMU"]: bias_col["FMU"] + 1],
                    scale=1.0,
                )

            def sig_epi(m, m_act, ps):
                t = tmp.tile([128, BC], F32, tag="sp")
                nc.scalar.activation(
                    t[0:Z_DIM, :], ps[0:Z_DIM, :], AF.Exp,
                    bias=bias_sb[0:Z_DIM, bias_col["FSIG"]: bias_col["FSIG"] + 1],
                    scale=1.0,
                )
                nc.scalar.activation(
                    sig[0:Z_DIM, :], t[0:Z_DIM, :], AF.Ln, bias=1.0, scale=1.0
                )
                nc.scalar.activation(
                    lsig[0:Z_DIM, :], sig[0:Z_DIM, :], AF.Ln, bias=0.0, scale=1.0
                )

            apply_layer("FMU", we["FMU"], e4, mu_epi)
            apply_layer("FSIG", we["FSIG"], e4, sig_epi)

        # resident generator weights (after encoder pools released)
        wgen = ctx.enter_context(tc.tile_pool(name="wgen", bufs=1))
        strm = ctx.enter_context(tc.tile_pool(name="strm", bufs=3))
        for nm in RES:
            w_sb[nm] = wgen.tile([128, packs[nm].ncols * 128], BF,
                                 tag=f"w{nm}", name=f"w{nm}")
            nc.sync.dma_start(w_sb[nm][:], dram[nm])

        # z0 = mu + sig*eps_z ; p0 = sqrt(T0)*eps_p (prescaled on host)
        epz = zp.tile([Z_DIM, BC], F32, tag="eps")
        nc.sync.dma_start(epz[:], d_epz)
        z_cur = zp.tile([Z_DIM, BC], F32, tag="z")
        nc.vector.tensor_tensor(z_cur[:], sig[0:Z_DIM, :], epz[:], ALU.mult)
        nc.vector.tensor_tensor(z_cur[:], z_cur[:], mu[0:Z_DIM, :], ALU.add)
        p_cur = zp.tile([Z_DIM, BC], F32, tag="p")
        nc.sync.dma_start(p_cur[:], d_epp)

        # ---------------- dU evaluation ----------------
        def eval_dU(z_in, last):
            zb = zp.tile([Z_DIM, BC], BF, tag="zb")
            nc.scalar.activation(zb[:], z_in[:], AF.Copy)
            apply_layer("G1", w_sb["G1"], zb, softplus_epi("G1", a1), k_rows=Z_DIM)
            apply_layer("G2", w_sb["G2"], a1, softplus_epi("G2", a2))
            apply_layer("D1", w_sb["D1"], a2, softplus_epi("D1", a3))
            apply_layer("D2", w_sb["D2"], a3, softplus_epi("D2", a4))

            c0 = bias_col["D3"]

            def d3_epi(m, m_act, ps):
                nc.scalar.activation(
                    eL[0:m_act, m * BC: m * BC + BC], ps[0:m_act, :], AF.Exp,
                    bias=bias_sb[0:m_act, c0 + m: c0 + m + 1], scale=-1.0,
                )
                if last:
                    nc.scalar.activation(
                        logit[0:m_act, m * BC: m * BC + BC], ps[0:m_act, :],
                        AF.Identity,
                        bias=bias_sb[0:m_act, c0 + m: c0 + m + 1], scale=1.0,
                    )
                    t = tmp.tile([128, BC], F32, tag="sp")
                    nc.scalar.activation(
                        t[0:m_act, :], ps[0:m_act, :], AF.Exp,
                        bias=bias_sb[0:m_act, c0 + m: c0 + m + 1], scale=1.0,
                    )
                    nc.scalar.activation(
                        spl[0:m_act, m * BC: m * BC + BC], t[0:m_act, :],
                        AF.Ln, bias=1.0, scale=1.0,
                    )

            apply_layer("D3", w_sb["D3"], a4, d3_epi)

            # seed = 1/(1+eL) - x
            for m in range(7):
                sl = slice(m * BC, (m + 1) * BC)
                u = tmp.tile([128, BC], F32, tag="sd1")
                nc.vector.tensor_scalar_add(u[:], eL[:, sl], 1.0)
                nc.vector.reciprocal(u[:], u[:])
                nc.vector.tensor_tensor(sd[:, sl], u[:], x_sb[:, sl], ALU.subtract)

            # e_i = exp(-a_i) in place; then gp_i = (e_i - 1) * g_i into a_i
            for t, n in ((a4, 25), (a3, 13), (a2, 13), (a1, 4)):
                nc.scalar.activation(t[:, 0: n * BC], t[:, 0: n * BC], AF.Exp,
                                     scale=-1.0)
            apply_layer("D3T", w_sb["D3T"], sd, grad_site_epi(a4, a4))
            stream_layer("D2T", a4, grad_site_epi(a3, a3))
            stream_layer("D1T", a3, grad_site_epi(a2, a2))
            apply_layer("G2T", w_sb["G2T"], a2, grad_site_epi(a1, a1))

            dU = zp.tile([Z_DIM, BC], F32, tag="dU")

            def gz_epi(m, m_act, ps):
                nc.vector.tensor_tensor(
                    dU[:], ps[0:Z_DIM, :], z_in[:], ALU.add
                )

            apply_layer("G1T", w_sb["G1T"], a1, gz_epi)
            return dU

        LF = lf_sb[0:Z_DIM, 0:1]
        NHLF = lf_sb[0:Z_DIM, 1:2]

        dU_cur = eval_dU(z_cur, last=False)
        for k in range(K_LF):
            last = k == K_LF - 1
            ph = zp.tile([Z_DIM, BC], F32, tag="ph")
            nc.vector.scalar_tensor_tensor(
                ph[:], dU_cur[:], NHLF, p_cur[:], ALU.mult, ALU.add
            )
            z_new = zp.tile([Z_DIM, BC], F32, tag="z")
            nc.vector.scalar_tensor_tensor(
                z_new[:], ph[:], LF, z_cur[:], ALU.mult, ALU.add
            )
            dU_new = eval_dU(z_new, last=last)
            p_new = zp.tile([Z_DIM, BC], F32, tag="p")
            nc.vector.scalar_tensor_tensor(
                p_new[:], dU_new[:], NHLF, ph[:], ALU.mult, ALU.add
            )
            nc.vector.tensor_scalar_mul(p_new[:], p_new[:], float(alphas[k]))
            z_cur, p_cur, dU_cur = z_new, p_new, dU_new

        # ell = sum_f x*logit - softplus(logit)
        for m in range(7):
            sl = slice(m * BC, (m + 1) * BC)
            nc.vector.tensor_tensor(logit[:, sl], x_sb[:, sl], logit[:, sl],
                                    ALU.mult)
            nc.vector.tensor_tensor(logit[:, sl], logit[:, sl], spl[:, sl],
                                    ALU.subtract)
        psr = pred.tile([128, BC], F32, tag="red")
        for m in range(7):
            nc.tensor.matmul(
                psr[0:1, :], ones[:, 0:1], logit[:, m * BC:(m + 1) * BC],
                start=(m == 0), stop=(m == 6),
            )
        nc.scalar.activation(ell[0:1, :], psr[0:1, :], AF.Copy)

        nc.sync.dma_start(d_out[0:1, :], ell[0:1, :])
        nc.sync.dma_start(d_out[1:21, :], z_cur[:])
        nc.sync.dma_start(d_out[21:41, :], p_cur[:])
        nc.sync.dma_start(d_out[41:61, :], lsig[0:Z_DIM, :])

    nc.compile()
    return nc


# ----------------------------------------------------------------------------
# public entry
# ----------------------------------------------------------------------------

def kernel(**ins):
    return _execute(ins)[1]


def _execute(ins, trace=False):
    from concourse import bass_utils

    key = "k"
    if key not in _CACHE:
        mats, biases = _build_mats(ins)
        packs = {nm: LayerPack(nm, m) for nm, m in mats.items()}

        # bias arena: one fp32 column per (layer, m-tile)
        order = ["A1", "A2", "A3", "FC1", "FMU", "FSIG", "G1", "G2", "D1",
                 "D2", "D3"]
        bias_col = {}
        cols = []
        for nm in order:
            bias_col[nm] = len(cols)
            b = biases[nm]
            Mt = packs[nm].Mt
            for m in range(Mt):
                c = np.zeros(128, np.float32)
                seg = b[m * 128:(m + 1) * 128]
                c[: len(seg)] = seg
                cols.append(c)
        biases_vec = np.stack(cols, axis=1)

        T0 = 1.0 + float(np.exp(np.float32(ins["T0_reparam"])))
        kv = np.arange(1, K_LF + 1, dtype=np.float64)
        ts_ = (1.0 - T0) * kv ** 2 / (K_LF * K_LF) + T0
        tsm = (1.0 - T0) * (kv - 1.0) ** 2 / (K_LF * K_LF) + T0
        alphas = np.sqrt(ts_ / tsm)
        lf = (1.0 / (1.0 + np.exp(-np.float64(ins["lf_reparam"])))) * MAX_LF
        scal = {"alphas": [float(a) for a in alphas], "bias_col": bias_col,
                "T0": T0, "lf": lf.astype(np.float32)}

        nc = _build_kernel(packs, biases_vec, scal)
        _CACHE[key] = (nc, packs, biases_vec, scal)
    nc, packs, biases_vec, scal = _CACHE[key]

    # ---- per-core input maps ----
    x = np.asarray(ins["x"], np.float32).reshape(B_FULL, 784)
    xp = np.zeros((B_FULL, 7 * 128), np.float32)
    xp[:, :784] = x
    eps_z = np.asarray(ins["eps_z"], np.float32)
    eps_p = np.asarray(ins["eps_p"], np.float32) * np.sqrt(scal["T0"])
    lfv = np.zeros((Z_DIM, 2), np.float32)
    lfv[:, 0] = scal["lf"]
    lfv[:, 1] = -0.5 * scal["lf"]

    in_maps = []
    for c in range(N_CORES):
        sl = slice(c * BC, (c + 1) * BC)
        xc = xp[sl]  # [BC, 896]
        # xin[p, kt*BC + b] = x[b, kt*128 + p]
        xin = np.ascontiguousarray(
            xc.reshape(BC, 7, 128).transpose(2, 1, 0).reshape(128, 7 * BC)
        ).astype(BF16)
        m = {
            "xin": xin,
            "biases": biases_vec,
            "epz": np.ascontiguousarray(eps_z[sl].T),
            "epp": np.ascontiguousarray(eps_p[sl].T),
            "lfv": lfv,
        }
        for nm, pk in packs.items():
            m[f"w_{nm}"] = pk.arena
        in_maps.append(m)

    res = bass_utils.run_bass_kernel_spmd(
        nc, in_maps, core_ids=list(range(N_CORES)), trace=trace
    )

    total = 0.0
    for c in range(N_CORES):
        o = np.asarray(res.results[c]["out"], np.float64)
        ellv = o[0]
        zK = o[1:21]
        pK = o[21:41]
        ls = o[41:61]
        negkl = (-0.5 * (zK ** 2).sum(0) - 0.5 * (pK ** 2).sum(0)
                 + ls.sum(0) + Z_DIM)
        total += (ellv + negkl).sum()
    return res, np.float32(total / B_FULL)
